# revision 53
# baseline (speedup 1.0000x reference)
"""Self-attention kernel for Trainium2 (8 NeuronCores, batch-parallel).

Computes, per batch element b:
    S = x_b^T @ x_b            [N, N]   (x_b is [C, N])
    W = softmax(S, axis=-1)
    out_b = x_b @ W^T          [C, N]   (out[c, i] = sum_j W[i, j] x[c, j])

B=8 batch elements map one-to-one onto the 8 NeuronCores (data parallel,
no collectives).

Per-core algorithm (mixed fp8/bf16 compute, f32 accumulation):
  1. Load x f32 (HWDGE), cast on DVE to fp8e4 (for S) and bf16 (for the
     context matmul); DMA-xbar-transpose x_bf16 to get xT.
  2. Row shift c_i = ||x_i||^2 (col-sums of x^2 via PE matmuls against -1s);
     softmax is shift-invariant so any per-row shift that prevents exp
     overflow works, and the Gram diagonal dominates the row max here.
  3. Pass 1 (16 row tiles x 2 j-halves): S via fp8-DoubleRow PE matmuls
     (full c=256 contraction per op at 2 MACs/cell/cycle) -> ScalarE
     exp(S - c_i) with accum_out giving row sums Z for free -> unnormalized
     E (bf16) -> DMA-xbar transpose into ET (j on partitions). The fp8
     error cancels between the exp numerator and Z, so only the bf16
     context-matmul rounding (~0.3%) reaches the output.
  4. Zinv = 1/Z, broadcast along partitions via a tiny DRAM round trip
     (with an f32->bf16 cast fused into the broadcast DMA).
  5. Pass 2: out = xT^T @ ET accumulated over j in PSUM ([c, i] layout),
     in 1-bank output quarters whose matmuls are statically woven between
     pass-1 steps (the per-engine instruction order is fixed at trace time,
     so PE idle during the ACT-bound pass 1 must be filled explicitly).
  6. Unnormalized PSUM->SBUF copies free banks early; scaling by 1/Z is
     deferred until zbc lands (fused into the copy for the last quarter)
     and split across DVE and GpSimd.
"""

import numpy as np

import concourse.bass as bass
import concourse.tile as tile
from concourse import bacc, mybir
from concourse.bass_utils import run_bass_kernel_spmd
from concourse.masks import make_identity

B, C, N = 8, 256, 2048
P = 128
CK = C // P  # 2 chunks of the channel dim
NT = N // P  # 16 row tiles
FP32 = mybir.dt.float32
BF16 = mybir.dt.bfloat16
FP8 = mybir.dt.float8e4

H = N // 2  # 1024: j-half size for S tiles (2 PSUM banks each)
NBH = H // 512  # 2
Q = 512  # i-quarter width used for x-load chunking and deferred scales

# Output i-groups for pass 2: (start, width).
GROUPS = [(0, 512), (512, 512), (1024, 512), (1536, 512)]
# last row tile each group's ET columns depend on
GROUP_READY = [(s + w) // P - 1 for s, w in GROUPS]
N_FUSED_GROUPS = 1  # trailing groups get the normalization fused

# Filler tuning: max context matmuls woven after each pass-1 (it, h) step,
# and the tile slack required before a group's inputs are considered ready.
FILL_PER_HALF = 4
QUARTER_SLACK = 3


def build_attention(tc, out_d, x_d, zrow_d):
    nc = tc.nc
    from contextlib import ExitStack

    with ExitStack() as ctx:
        singles = ctx.enter_context(tc.tile_pool(name="singles", bufs=1))
        epool = ctx.enter_context(tc.tile_pool(name="epool", bufs=6))
        psum = ctx.enter_context(tc.tile_pool(name="psum", bufs=1, space="PSUM"))

        # ---- preload the exp table set so the first real exp doesn't pay it
        warm = singles.tile([P, 1], FP32)
        nc.vector.memset(warm, 0.0)
        nc.scalar.activation(
            out=warm, in_=warm, func=mybir.ActivationFunctionType.Exp
        )

        # ---- load x f32 via HWDGE (parallel rings, low first-byte latency)
        # then cast to bf16 on DVE, in (j-quarter, chunk) units so the first
        # S matmuls start early. (SWDGE cast-DMAs serialize ~1us apart.)
        x_f32 = singles.tile([P, CK, N], FP32)
        x_bf = singles.tile([P, CK, N], BF16)
        x_f8 = singles.tile([P, CK, N], FP8)
        xsq = singles.tile([P, CK, N], BF16)
        neg_ones = singles.tile([P, 1], BF16)
        nc.vector.memset(neg_ones, -1.0)
        negc = singles.tile([P, NT], FP32)
        # Per j-quarter: DMA f32, cast fp8 (S-matmul critical path), square
        # from f32 (negc critical path); bf16 casts (only needed for xT and
        # the context lhsT) are deferred into pass 1.
        def cast_f8(jq):
            for cc in range(CK):
                nc.vector.tensor_copy(
                    x_f8[:, cc, jq * Q : (jq + 1) * Q],
                    x_f32[:, cc, jq * Q : (jq + 1) * Q],
                )

        def negc_quarter(jq):
            for cc in range(CK):
                nc.vector.tensor_mul(
                    xsq[:, cc, jq * Q : (jq + 1) * Q],
                    x_f32[:, cc, jq * Q : (jq + 1) * Q],
                    x_f32[:, cc, jq * Q : (jq + 1) * Q],
                )
            negc_ps = psum.tile([P, Q], FP32, tag="o", bufs=4, name=f"negc_ps{jq}")
            for k in range(4):
                it = jq * 4 + k
                for cc in range(CK):
                    nc.tensor.matmul(
                        negc_ps[:, k : k + 1],
                        lhsT=xsq[:, cc, it * P : (it + 1) * P],
                        rhs=neg_ones,
                        start=(cc == 0),
                        stop=(cc == CK - 1),
                    )
            nc.vector.tensor_copy(negc[:, jq * 4 : (jq + 1) * 4], negc_ps[:, 0:4])

        # DVE prologue order matters (static per-engine streams): tile 0
        # needs fp8 of ALL quarters (its S row spans every column) plus
        # negc quarter 0, so those casts come first; the remaining xsq/negc
        # work follows.
        for jq in range(4):
            for cc in range(CK):
                nc.sync.dma_start(
                    out=x_f32[:, cc, jq * Q : (jq + 1) * Q],
                    in_=x_d[cc * P : (cc + 1) * P, jq * Q : (jq + 1) * Q],
                )
        cast_f8(0)
        negc_quarter(0)
        for jq in range(1, 4):
            cast_f8(jq)
        for jq in range(1, 4):
            negc_quarter(jq)

        xT = singles.tile([P, NT, C], BF16)
        Zh = singles.tile([P, NT, 2], FP32)
        ET = singles.tile([P, NT, N], BF16)
        zbc = singles.tile([P, N], BF16)
        out_sb = singles.tile([P, CK, N], FP32)

        # ---- pass-2 work queue: (g, cc, jt) in group-major order
        ctx_queue = [
            (g, cc, jt)
            for g in range(len(GROUPS))
            for cc in range(CK)
            for jt in range(NT)
        ]
        qi = 0  # queue position
        o_ps = {}

        def emit_ctx_mm():
            nonlocal qi
            g, cc, jt = ctx_queue[qi]
            qi += 1
            s, w = GROUPS[g]
            if jt == 0:
                o_ps[(g, cc)] = psum.tile(
                    [P, Q], FP32, tag="o", bufs=4, name=f"o_ps{g}_{cc}"
                )
            nc.tensor.matmul(
                o_ps[(g, cc)][:, 0:w],
                lhsT=xT[:, jt, cc * P : (cc + 1) * P],
                rhs=ET[:, jt, s : s + w],
                start=(jt == 0),
                stop=(jt == NT - 1),
            )
            if jt == NT - 1:
                if g < len(GROUPS) - N_FUSED_GROUPS:
                    # unnormalized copy frees the PSUM bank without waiting
                    # for the Z chain; scaled later
                    nc.vector.tensor_copy(
                        out_sb[:, cc, s : s + w], o_ps[(g, cc)][:, 0:w]
                    )
                else:
                    # trailing groups: zbc is ready by now; fuse normalization
                    nc.vector.tensor_mul(
                        out_sb[:, cc, s : s + w],
                        o_ps[(g, cc)][:, 0:w],
                        zbc[:, s : s + w],
                    )
                    nc.sync.dma_start(
                        out=out_d[cc * P : (cc + 1) * P, s : s + w],
                        in_=out_sb[:, cc, s : s + w],
                    )

        def ctx_available(it):
            if qi >= len(ctx_queue):
                return False
            g = ctx_queue[qi][0]
            return it >= GROUP_READY[g] + QUARTER_SLACK

        # ---- pass 1 with woven pass-2 filler
        for it in range(NT):
            for h in range(2):
                s_ps = psum.tile([P, H], FP32, tag="s", bufs=2, name=f"s_ps{it}_{h}")
                for nb in range(NBH):
                    # fp8 DoubleRow: both operands [K=128, 2, dim]; the pair
                    # dim contracts, giving the full c=256 reduction in one op
                    # at 2 MACs/cell/cycle.
                    j0 = h * H + nb * 512
                    nc.tensor.matmul(
                        s_ps[:, nb * 512 : (nb + 1) * 512],
                        lhsT=x_f8[:, :, it * P : (it + 1) * P],
                        rhs=x_f8[:, :, j0 : j0 + 512],
                        start=True,
                        stop=True,
                        perf_mode=mybir.MatmulPerfMode.DoubleRow,
                    )
                e_t = epool.tile([P, H], BF16, tag="e", name=f"e{it}_{h}")
                nc.scalar.activation(
                    out=e_t,
                    in_=s_ps,
                    func=mybir.ActivationFunctionType.Exp,
                    bias=negc[:, it : it + 1],
                    scale=1.0,
                    accum_out=Zh[:, it, h : h + 1],
                )
                nc.sync.dma_start_transpose(
                    out=ET[
                        :, h * (NT // 2) : (h + 1) * (NT // 2), it * P : (it + 1) * P
                    ],
                    in_=e_t,
                )
                for _ in range(FILL_PER_HALF):
                    if ctx_available(it):
                        emit_ctx_mm()
            if it == 1:
                # deferred bf16 casts (feed xT and the context lhsT only)
                for jq in range(4):
                    for cc in range(CK):
                        nc.vector.tensor_copy(
                            x_bf[:, cc, jq * Q : (jq + 1) * Q],
                            x_f32[:, cc, jq * Q : (jq + 1) * Q],
                        )
            if it == 2:
                # xT transposes traced here: late enough not to head-block
                # the HWDGE queue (they wait on all of x), early enough for
                # the first woven context matmuls
                for cc in range(CK):
                    nc.sync.dma_start_transpose(
                        out=xT[:, :, cc * P : (cc + 1) * P], in_=x_bf[:, cc, :]
                    )

        # ---- Zinv broadcast: Zh -> Z -> 1/Z -> row [16,128] -> DRAM -> [128,2048]
        Z = singles.tile([P, NT], FP32)
        nc.vector.tensor_add(Z, Zh[:, :, 0], Zh[:, :, 1])
        zinv = singles.tile([P, NT], FP32)
        nc.vector.reciprocal(zinv, Z)
        ident = singles.tile([P, P], FP32)
        make_identity(nc, ident)
        zt_ps = psum.tile([P, Q], FP32, tag="o", bufs=4)
        nc.tensor.transpose(zt_ps[0:NT, 0:P], zinv, ident)
        zrow = singles.tile([NT, P], FP32)
        nc.vector.tensor_copy(zrow, zt_ps[0:NT, 0:P])
        nc.sync.dma_start(out=zrow_d, in_=zrow)
        # broadcast along partitions with an f32->bf16 cast in the same
        # SWDGE DMA (halves the broadcast traffic; zinv precision ~0.4%)
        nc.gpsimd.dma_start(
            out=zbc, in_=zrow_d.rearrange("a b -> (a b)").partition_broadcast(P)
        )

        # ---- deferred scales for the leading groups, split across DVE and
        # GpSimd so the post-zbc burst halves in wall time; stores ride the
        # otherwise-idle ACT HWDGE ring
        def scale_and_store(g, cc):
            s, w = GROUPS[g]
            sl = out_sb[:, cc, s : s + w]
            eng = nc.vector if cc == 0 else nc.gpsimd
            eng.tensor_mul(sl, sl, zbc[:, s : s + w])
            nc.sync.dma_start(out=out_d[cc * P : (cc + 1) * P, s : s + w], in_=sl)

        n_deferred = len(GROUPS) - N_FUSED_GROUPS
        scaled = set()
        while qi < len(ctx_queue):
            g_done = qi // (2 * NT) - 1  # last fully emitted group
            for g in range(min(g_done + 1, n_deferred)):
                for cc in range(CK):
                    if (g, cc) not in scaled:
                        scaled.add((g, cc))
                        scale_and_store(g, cc)
            emit_ctx_mm()
        for g in range(n_deferred):
            for cc in range(CK):
                if (g, cc) not in scaled:
                    scale_and_store(g, cc)


def build_nc(reps: int = 1):
    nc = bacc.Bacc(
        "TRN2",
        target_bir_lowering=False,
        debug=False,
        enable_asserts=False,
        num_devices=B,
    )
    x_d = nc.dram_tensor("x", [C, N], FP32, kind="ExternalInput").ap()
    out_d = nc.dram_tensor("out", [C, N], FP32, kind="ExternalOutput").ap()
    zrow_d = nc.dram_tensor("zrow_scratch", [NT, P], FP32).ap()
    with tile.TileContext(nc) as tc:
        for _ in range(reps):
            build_attention(tc, out_d, x_d, zrow_d)
    nc.compile()
    return nc


_NC_CACHE = None


def _get_nc():
    global _NC_CACHE
    if _NC_CACHE is None:
        _NC_CACHE = build_nc()
    return _NC_CACHE


def kernel(x: np.ndarray) -> np.ndarray:
    """x: [8, 256, 2048] float32 -> [8, 256, 2048] float32."""
    x = np.asarray(x, dtype=np.float32)
    assert x.shape == (B, C, N), x.shape
    nc = _get_nc()
    in_maps = [{"x": np.ascontiguousarray(x[b])} for b in range(B)]
    res = run_bass_kernel_spmd(nc, in_maps, core_ids=list(range(B)))
    return np.stack([res.results[b]["out"] for b in range(B)], axis=0)


if __name__ == "__main__":
    import jax

    key = jax.random.key(0)
    xs = np.asarray(
        jax.random.normal(key, (B, C, N), dtype=np.float32), dtype=np.float32
    )
    out = kernel(xs)
    print("out", out.shape, out.dtype)
